# revision 6
# baseline (speedup 1.0000x reference)
"""Data-parallel linear layer (x @ W.T + bias) on 8 TRN2 NeuronCores.

Shard x over batch: each core computes a (1024 x 2048) @ (2048 x 2048).T
matmul in float32r (full-rate fp32 PE mode), bias added on DVE.

Per-core schedule: 4 output-column blocks (n) of 512.
 - n=0,1: k-major (stream k-slabs of x and W; PSUM groups for all 8 m
   interleave per k) -- degrades gracefully while the 12 MiB n=0 input
   crunch is in flight.
 - n=2,3: m-major (16 k-contiguous matmuls per PSUM group) -- spreads
   the DVE bias-add drains and output DMAs evenly, so the kernel tail
   is one drain, not eight.
DMAs: inputs ride HWDGE (nc.sync), outputs ride SWDGE (nc.gpsimd) so
weight prefetch never queues behind output writes. W loads are k-paired
[128,2,512] and x slabs single [128,1024] to minimize DMA/semaphore
count (boot + drain EVSEM storms scale with it).
"""
import numpy as np

import concourse.bass as bass  # noqa: F401
import concourse.mybir as mybir
import concourse.tile as tile
from concourse import bacc, bass_utils

B, IN, OUT = 8192, 2048, 2048
NCORES = 8
BS = B // NCORES      # 1024 batch rows per core
P = 128               # partition dim
NFREE = 512           # fp32 moving-operand max / one PSUM bank
KT = IN // P          # 16 contraction tiles
MT = BS // P          # 8 output-row tiles per core
NT = OUT // NFREE     # 4 output-col tiles
XCHUNK_SLABS = 2      # first k-slabs loaded as 2x[128,512] for fast start

F32 = mybir.dt.float32
F32R = mybir.dt.float32r

TRACE = False
LAST_EXEC_NS = None

_NC_CACHE = {}


def _build():
    if "nc" in _NC_CACHE:
        return _NC_CACHE["nc"]
    nc = bacc.Bacc("TRN2", target_bir_lowering=False, debug=False)
    xT = nc.dram_tensor("xT", [IN, BS], F32R, kind="ExternalInput")
    wT = nc.dram_tensor("wT", [IN, OUT], F32R, kind="ExternalInput")
    bias_b = nc.dram_tensor("bias_b", [P, OUT], F32, kind="ExternalInput")
    out = nc.dram_tensor("out", [BS, OUT], F32, kind="ExternalOutput")

    xT_ap = xT.ap()
    out_ap = out.ap()
    # [k2, p, a, o]: element = wT[k2*256 + a*128 + p, o]
    wT_r = wT.ap().rearrange("(b a p) o -> b p a o", a=2, p=P)

    with tile.TileContext(nc) as tc:
        with tc.tile_pool(name="xp", bufs=1) as xp, \
             tc.tile_pool(name="wp", bufs=1) as wp, \
             tc.tile_pool(name="bp", bufs=1) as bp, \
             tc.tile_pool(name="op", bufs=8) as op, \
             tc.tile_pool(name="pp", bufs=8, space="PSUM") as pp:
            bias_sb = bp.tile([P, OUT], F32)
            x_sb = [None] * KT          # k -> tile or (chunk0, chunk1)
            w_sb = [[None] * (KT // 2) for _ in range(NT)]

            def emit_x_dma(k):
                if k < XCHUNK_SLABS:
                    cs = []
                    for c in range(2):
                        t = xp.tile([P, NFREE], F32R, tag="xc", bufs=4,
                                    name=f"x_{k}_{c}")
                        nc.sync.dma_start(
                            t[:], xT_ap[k * P:(k + 1) * P,
                                        c * NFREE:(c + 1) * NFREE])
                        cs.append(t)
                    x_sb[k] = tuple(cs)
                else:
                    t = xp.tile([P, BS], F32R, tag="x",
                                bufs=KT - XCHUNK_SLABS, name=f"x_{k}")
                    nc.sync.dma_start(t[:], xT_ap[k * P:(k + 1) * P, :])
                    x_sb[k] = t

            def emit_w_dma(n, k2):
                t = wp.tile([P, 2, NFREE], F32R, tag="w", bufs=16,
                            name=f"w_{n}_{k2}")
                nc.sync.dma_start(
                    t[:], wT_r[k2][:, :, n * NFREE:(n + 1) * NFREE])
                w_sb[n][k2] = t

            def x_slice(k, m):
                if k < XCHUNK_SLABS:
                    c = m // (MT // 2)
                    off = (m % (MT // 2)) * P
                    return x_sb[k][c][:, off:off + P]
                return x_sb[k][:, m * P:(m + 1) * P]

            def mm(n, k, m, ps_m):
                nc.tensor.matmul(
                    ps_m[:],
                    x_slice(k, m),
                    w_sb[n][k // 2][:, k % 2, :],
                    start=(k == 0),
                    stop=(k == KT - 1),
                )

            def drain(n, m, ps_m):
                ot = op.tile([P, NFREE], F32, tag="o", name=f"o_{n}_{m}")
                nc.vector.tensor_add(
                    ot[:], ps_m[:], bias_sb[:, n * NFREE:(n + 1) * NFREE])
                nc.gpsimd.dma_start(
                    out_ap[m * P:(m + 1) * P,
                           n * NFREE:(n + 1) * NFREE], ot[:])

            for n in range(NT):
                for k2 in range(KT // 2):
                    if n == 0:
                        emit_x_dma(2 * k2)
                        emit_x_dma(2 * k2 + 1)
                    emit_w_dma(n, k2)
                    if n == 0 and k2 == KT // 4:
                        # bias needed only at the first drain; keep it
                        # out of the startup DMA crunch
                        nc.sync.dma_start(bias_sb[:], bias_b.ap())

                ps = [pp.tile([P, NFREE], F32, tag="ps", name=f"ps_{n}_{m}")
                      for m in range(MT)]
                if n < 2:
                    for k in range(KT):
                        for m in range(MT):
                            mm(n, k, m, ps[m])
                    for m in range(MT):
                        drain(n, m, ps[m])
                else:
                    for m in range(MT):
                        for k in range(KT):
                            mm(n, k, m, ps[m])
                        drain(n, m, ps[m])
    nc.compile()
    _NC_CACHE["nc"] = nc
    return nc


def kernel(x: np.ndarray, weight: np.ndarray, bias: np.ndarray) -> np.ndarray:
    global LAST_EXEC_NS
    x = np.asarray(x, dtype=np.float32)
    weight = np.asarray(weight, dtype=np.float32)
    bias = np.asarray(bias, dtype=np.float32)

    xT = np.ascontiguousarray(x.T)            # [IN, B]
    wT = np.ascontiguousarray(weight.T)       # [IN, OUT]
    bias_b = np.ascontiguousarray(
        np.broadcast_to(bias[None, :], (P, OUT)), dtype=np.float32)

    in_maps = [
        {
            "xT": np.ascontiguousarray(xT[:, c * BS:(c + 1) * BS]),
            "wT": wT,
            "bias_b": bias_b,
        }
        for c in range(NCORES)
    ]

    nc = _build()
    res = bass_utils.run_bass_kernel_spmd(
        nc, in_maps, core_ids=list(range(NCORES)), trace=TRACE)
    LAST_EXEC_NS = res.exec_time_ns

    return np.concatenate([r["out"] for r in res.results], axis=0)
